# revision 17
# baseline (speedup 1.0000x reference)
"""DeepSpeed-style fused MLP (residual-add + LayerNorm + GEMM1 + GELU + GEMM2
+ bias/residual add) on 8 Trainium2 NeuronCores.

Tensor-parallel over the intermediate dim (DeepSpeed style):
  - Each core LayerNorms 1/8 of the tokens (bn_stats/bn_aggr for the moments,
    bias-add on GpSimd), PE-transposes the normalized activations to [H, tok]
    bf16 staged fully in SBUF, one 4.2MB store, one AllGather shares them.
  - attn_nw/attn_nb and bias+output_b are folded into the weights/fa_sum on
    the host (exact).
  - Per 512-token group: GEMM1 (bf16, fp32 accum) -> bias+gelu on ScalarE ->
    GEMM2 -> bf16 partial (PSUM evacuation alternating ScalarE/VectorE) -> ReduceScatter (bf16 payload, pipelined behind
    the next group's compute) -> owning core adds residuals.
  - DMA is spread over three queues to avoid head-of-line blocking: weight
    loads on the SP HWDGE ring, stores + final-stage traffic on the ACT
    HWDGE ring, activation (lnT) prefetch on the GpSimd SWDGE ring, issued
    one group ahead.

Self-contained: hardcodes the problem shapes (B=2, S=2048, H=4096, I=16384).
"""
import numpy as np
import ml_dtypes

BF16_NP = ml_dtypes.bfloat16

B, S, H, I = 2, 2048, 4096, 16384
T = B * S
NC = 8
P = 128
NT = 512                 # tokens per group
G = T // NT
TPC = T // NC
ISH = I // NC
IC = ISH // P
HC = H // P
FN = 512
HN = H // FN
OWN = NT // NC
EPS = 1e-12

_BUILD_CACHE = {}


def _build(t, h, i_dim, stage='full', reps=1):
    import concourse.bass as bass
    import concourse.mybir as mybir
    import concourse.tile as tile
    from concourse import bacc
    from concourse.bass import ts
    from concourse.masks import make_identity
    from contextlib import ExitStack

    f32 = mybir.dt.float32
    bf16 = mybir.dt.bfloat16
    A = mybir.ActivationFunctionType
    OP = mybir.AluOpType

    nt = NT
    g_cnt = t // nt
    tpc = t // NC
    ish = i_dim // NC
    ic_cnt = ish // P
    hc_cnt = h // P
    fn = min(FN, h)
    hn_cnt = h // fn
    own = nt // NC
    fch = min(512, h)
    nst = h // 512           # bn_stats chunks
    rg = [list(range(NC))]

    nc = bacc.Bacc(trn_type="TRN2", num_devices=NC)

    xin = nc.dram_tensor("xin", (tpc, h), f32, kind="ExternalInput")
    xres = nc.dram_tensor("xres", (tpc, h), f32, kind="ExternalInput")
    ln_bias = nc.dram_tensor("ln_bias", (h,), f32, kind="ExternalInput")
    wiT = nc.dram_tensor("wiT", (ic_cnt, P, hc_cnt * P), bf16,
                         kind="ExternalInput")
    bi = nc.dram_tensor("bi", (P, ic_cnt), f32, kind="ExternalInput")
    woT = nc.dram_tensor("woT", (hn_cnt, P, ic_cnt * fn), bf16,
                         kind="ExternalInput")
    fa_sum = nc.dram_tensor("fa_sum", (g_cnt * own, h), f32,
                            kind="ExternalInput")
    out_ext = nc.dram_tensor("out", (g_cnt * own, h), f32,
                             kind="ExternalOutput")

    def bcast(ap, parts):
        return bass.AP(tensor=ap.tensor, offset=ap.offset,
                       ap=[[0, parts]] + list(ap.ap))

    with tile.TileContext(nc) as tc, ExitStack() as ctx:
        consts = ctx.enter_context(tc.tile_pool(name="consts", bufs=1))
        dram = ctx.enter_context(tc.tile_pool(name="dram", bufs=1,
                                              space="DRAM"))
        dram2 = ctx.enter_context(tc.tile_pool(name="dram2", bufs=4,
                                               space="DRAM"))

        bi_sb = consts.tile([P, ic_cnt], f32)
        nc.sync.dma_start(bi_sb[:], bi[:])
        ident = consts.tile([P, P], bf16)
        make_identity(nc, ident[:])
        eps_t = consts.tile([P, 1], f32)
        nc.vector.memset(eps_t[:], EPS)

        for rep in range(reps):
            ag_in = dram.tile([h, tpc], bf16)
            ag_out = dram.tile([NC * h, tpc], bf16, addr_space="Shared")

            # ---- Stage 1: local LayerNorm (+ attn bias) and PE transpose ----
            with tc.tile_pool(name="lnp", bufs=2) as lnp, \
                 tc.tile_pool(name="repp", bufs=1) as repp, \
                 tc.tile_pool(name="pstr", bufs=4, space="PSUM") as pstr, \
                 tc.tile_pool(name="stgp", bufs=1) as stgp:
                rep_bias = repp.tile([P, h], f32)
                nc.sync.dma_start(rep_bias[:], bcast(ln_bias[:], P))
                lnbT = stgp.tile([P, hc_cnt, tpc], bf16)
                for tb in range(tpc // P):
                    x_t = lnp.tile([P, h], f32, tag="x_t")
                    nc.sync.dma_start(x_t[:], xin[ts(tb, P)])
                    r_t = lnp.tile([P, h], f32, tag="r_t")
                    nc.sync.dma_start(r_t[:], xres[ts(tb, P)])
                    ra = lnp.tile([P, h], f32, tag="ra")
                    ra2 = ra[:]
                    nc.vector.tensor_add(ra2, x_t[:], r_t[:])
                    nc.gpsimd.tensor_add(ra2, ra2, rep_bias[:])
                    st6 = lnp.tile([P, nst, 6], f32, tag="st6")
                    for c in range(nst):
                        nc.vector.bn_stats(st6[:, c, :], ra[:, ts(c, 512)])
                    mv = lnp.tile([P, 2], f32, tag="mv")
                    nc.vector.bn_aggr(mv[:], st6[:])
                    rstd = lnp.tile([P, 1], f32, tag="rstd")
                    nc.scalar.activation(rstd[:], mv[:, 1:2], A.Sqrt,
                                         bias=eps_t[:])
                    nc.vector.reciprocal(rstd[:], rstd[:])
                    lnb = lnp.tile([P, h], bf16, tag="lnb")
                    nc.vector.tensor_scalar(lnb[:], ra[:], mv[:, 0:1], rstd[:],
                                            op0=OP.subtract, op1=OP.mult)
                    for hcb in range(hc_cnt):
                        ps_tr = pstr.tile([P, P], bf16, tag="ps_tr")
                        nc.tensor.transpose(ps_tr[:], lnb[:, ts(hcb, P)],
                                            ident[:])
                        nc.scalar.copy(lnbT[:, hcb, ts(tb, P)], ps_tr[:])
                nc.scalar.dma_start(
                    ag_in[:].rearrange("(hc p) t -> p hc t", p=P), lnbT[:])

            if stage in ('ag', 'full'):
                nc.gpsimd.collective_compute(
                    "AllGather", mybir.AluOpType.bypass, replica_groups=rg,
                    ins=[ag_in[:].opt()], outs=[ag_out[:].opt()])

            # ---- Stage 2: per-PAIR of groups, sharing each weight-tile load:
            #      GEMM1 -> gelu -> GEMM2 -> RS -> final.  Weight HBM traffic
            #      halves vs per-group reload (134MB vs 268MB per core). ----
            if stage not in ('ln', 'ag'):
                gpp = 2 if g_cnt % 2 == 0 else 1   # groups per pair
                np_cnt = g_cnt // gpp
                with tc.tile_pool(name="lntp", bufs=gpp + 1) as lntp, \
                   tc.tile_pool(name="intp", bufs=gpp) as intp, \
                   tc.tile_pool(name="w1p", bufs=2) as w1p, \
                   tc.tile_pool(name="w2p", bufs=2) as w2p, \
                   tc.tile_pool(name="obp", bufs=4) as obp, \
                   tc.tile_pool(name="fap", bufs=2) as fap, \
                   tc.tile_pool(name="ps1", bufs=4, space="PSUM") as ps1p, \
                   tc.tile_pool(name="ps2", bufs=4, space="PSUM") as ps2p:
                  lnT_tiles = {}

                  def load_lnT(g):
                      # issued on the SWDGE (gpsimd) ring so it never queues
                      # behind the SP-ring weight loads or ACT-ring stores
                      lnT = lntp.tile([P, hc_cnt, nt], bf16, tag="lnT")
                      for j in range(nt // tpc):
                          blk = g * (nt // tpc) + j
                          nc.gpsimd.dma_start(
                              lnT[:, :, ts(j, tpc)],
                              ag_out[blk * h:(blk + 1) * h, :].rearrange(
                                  "(hc p) t -> p hc t", p=P))
                      lnT_tiles[g] = lnT

                  for g in range(gpp):
                      load_lnT(g)
                  for p in range(np_cnt):
                      g0 = p * gpp
                      if g0 + gpp < g_cnt:
                          load_lnT(g0 + gpp)
                      lnTs = [lnT_tiles.pop(g0 + gg) for gg in range(gpp)]
                      interTs = [intp.tile([P, ic_cnt, nt], bf16, tag="interT",
                                           name=f"interT{gg}")
                                 for gg in range(gpp)]
                      # GEMM1: C1T[i, t] += wiT[h, i].T @ lnT[h, t]
                      for ic in range(ic_cnt):
                          w1 = w1p.tile([P, hc_cnt * P], bf16, tag="w1")
                          nc.sync.dma_start(w1[:], wiT[ic])
                          for gg in range(gpp):
                              ps = ps1p.tile([P, nt], f32, tag="ps")
                              for hcb in range(hc_cnt):
                                  nc.tensor.matmul(ps[:], w1[:, ts(hcb, P)],
                                                   lnTs[gg][:, hcb, :],
                                                   start=(hcb == 0),
                                                   stop=(hcb == hc_cnt - 1))
                              nc.scalar.activation(interTs[gg][:, ic, :], ps[:],
                                                   A.Gelu_apprx_tanh,
                                                   bias=bi_sb[:, ic:ic + 1])
                      if g0 + gpp + 1 < g_cnt:
                          load_lnT(g0 + gpp + 1)
                      if stage == 'g1':
                          continue
                      # GEMM2: out[t, hblk] += interT[i, t].T @ woT[i, hblk]
                      rs_ins = [dram2.tile([nt, h], bf16, tag="rs_in",
                                           name=f"rs_in{gg}")
                                for gg in range(gpp)]

                      def g2_block(gg, hn, w2):
                          for tsb in range(nt // P):
                              ps2 = ps2p.tile([P, fn], f32, tag="ps2",
                                              name="ps2")
                              for ic in range(ic_cnt):
                                  nc.tensor.matmul(ps2[:],
                                                   interTs[gg][:, ic, ts(tsb, P)],
                                                   w2[:, ts(ic, fn)],
                                                   start=(ic == 0),
                                                   stop=(ic == ic_cnt - 1))
                              if stage == 'g2mm':
                                  continue
                              ob = obp.tile([P, fn], bf16, tag="ob", name="ob")
                              if tsb % 2 == 0:
                                  nc.scalar.copy(ob[:], ps2[:])
                              else:
                                  nc.vector.tensor_copy(ob[:], ps2[:])
                              if stage == 'g2cp':
                                  continue
                              nc.scalar.dma_start(
                                  rs_ins[gg][ts(tsb, P), ts(hn, fn)], ob[:])

                      def rs_final(gg):
                          if stage in ('g2mm', 'g2cp', 'nors'):
                              return
                          g = g0 + gg
                          rs_out = dram2.tile([own, h], bf16, tag="rs_out",
                                              name="rs_out")
                          nc.gpsimd.collective_compute(
                              "ReduceScatter", mybir.AluOpType.add,
                              replica_groups=rg,
                              ins=[rs_ins[gg][:].opt()], outs=[rs_out[:].opt()])
                          # final: out = rs_out + fa_sum
                          for ch in range(h // fch):
                              fo = fap.tile([own, fch], bf16, tag="fo",
                                            name="fo")
                              nc.scalar.dma_start(fo[:], rs_out[:, ts(ch, fch)])
                              fi = fap.tile([own, fch], f32, tag="fi",
                                            name="fi")
                              nc.scalar.dma_start(fi[:],
                                                  fa_sum[ts(g, own), ts(ch, fch)])
                              nc.vector.tensor_add(fi[:], fi[:], fo[:])
                              nc.scalar.dma_start(
                                  out_ext[ts(g, own), ts(ch, fch)], fi[:])

                      if p < np_cnt - 1:
                          # shared w2 across the pair
                          for hn in range(hn_cnt):
                              w2 = w2p.tile([P, ic_cnt * fn], bf16, tag="w2")
                              nc.scalar.dma_start(w2[:], woT[hn])
                              for gg in range(gpp):
                                  g2_block(gg, hn, w2)
                          for gg in range(gpp):
                              rs_final(gg)
                      else:
                          # last pair: per-group w2 reload so group g0's RS
                          # overlaps the final group's GEMM2 instead of
                          # serializing after it
                          for gg in range(gpp):
                              for hn in range(hn_cnt):
                                  w2 = w2p.tile([P, ic_cnt * fn], bf16,
                                                tag="w2", name="w2")
                                  nc.scalar.dma_start(w2[:], woT[hn])
                                  g2_block(gg, hn, w2)
                              rs_final(gg)
    nc.finalize()
    return nc


def get_nc(t=T, h=H, i_dim=I, stage='full', reps=1):
    key = (t, h, i_dim, stage, reps)
    if key not in _BUILD_CACHE:
        _BUILD_CACHE[key] = _build(t, h, i_dim, stage, reps)
    return _BUILD_CACHE[key]


def prep_in_maps(input, residual, bias, attn_nw, attn_nb, inter_w, inter_b,
                 output_w, output_b, t=T, h=H, i_dim=I):
    nt = NT
    g_cnt = t // nt
    tpc = t // NC
    ish = i_dim // NC
    ic_cnt = ish // P
    hc_cnt = h // P
    fn = min(FN, h)
    hn_cnt = h // fn
    own = nt // NC

    x2 = np.ascontiguousarray(np.asarray(input, dtype=np.float32).reshape(t, h))
    r2 = np.ascontiguousarray(np.asarray(residual, dtype=np.float32).reshape(t, h))
    bias = np.asarray(bias, dtype=np.float32)
    nw = np.asarray(attn_nw, dtype=np.float32)
    nb = np.asarray(attn_nb, dtype=np.float32)
    wi = np.asarray(inter_w, dtype=np.float32)
    ib = np.asarray(inter_b, dtype=np.float32)
    wo = np.asarray(output_w, dtype=np.float32)
    ob = np.asarray(output_b, dtype=np.float32)

    bsum = bias + ob
    x4 = x2.reshape(g_cnt, NC, own, h)
    r4 = r2.reshape(g_cnt, NC, own, h)

    in_maps = []
    for c in range(NC):
        lo, hi = c * ish, (c + 1) * ish
        wi_c = wi[lo:hi]                       # [ish, h]
        wiT_eff = (wi_c * nw[None, :]).T       # [h, ish]
        w1 = np.ascontiguousarray(
            wiT_eff.reshape(hc_cnt, P, ic_cnt, P).transpose(2, 1, 0, 3)
            .reshape(ic_cnt, P, hc_cnt * P)).astype(BF16_NP)
        bi_eff = ib[lo:hi] + nb @ wi_c.T       # [ish]
        bi_c = np.ascontiguousarray(bi_eff.reshape(ic_cnt, P).T)
        woT_c = wo[:, lo:hi].T                 # [ish, h]
        w2 = np.ascontiguousarray(
            woT_c.reshape(ic_cnt, P, hn_cnt, fn).transpose(2, 1, 0, 3)
            .reshape(hn_cnt, P, ic_cnt * fn)).astype(BF16_NP)
        in_maps.append({
            "xin": np.ascontiguousarray(x2[c * tpc:(c + 1) * tpc]),
            "xres": np.ascontiguousarray(r2[c * tpc:(c + 1) * tpc]),
            "ln_bias": bias,
            "wiT": w1,
            "bi": bi_c,
            "woT": w2,
            "fa_sum": np.ascontiguousarray(
                x4[:, c].reshape(g_cnt * own, h)
                + r4[:, c].reshape(g_cnt * own, h) + bsum[None, :]),
        })
    return in_maps


def assemble(results, t=T, h=H):
    g_cnt = t // NT
    own = NT // NC
    out = np.empty((g_cnt, NC, own, h), dtype=np.float32)
    for c in range(NC):
        out[:, c] = results[c]["out"].reshape(g_cnt, own, h)
    return out.reshape(t, h)


def run(inputs, t=T, h=H, i_dim=I, trace=False, stage='full'):
    from concourse import bass_utils
    nc = get_nc(t, h, i_dim, stage)
    in_maps = prep_in_maps(**inputs, t=t, h=h, i_dim=i_dim)
    res = bass_utils.run_bass_kernel_spmd(
        nc, in_maps, core_ids=list(range(NC)), trace=trace)
    out = assemble(res.results, t=t, h=h)
    return out, res


def kernel(**inputs):
    out, _ = run(inputs)
    return out.reshape(B, S, H).astype(np.float32)

